# revision 8
# baseline (speedup 1.0000x reference)
"""PointPillarScatter on 8 Trainium2 NeuronCores.

out[b, c, y*NX+x] = pillar_features[p, c] for each pillar p with coords
(b, 0, y, x); duplicate (b,y,x) resolved last-pillar-wins; rest zeros.

Sharding: data-parallel over batch — core b handles batch element b.

Per-core device algorithm (gather formulation of the scatter):
  1. Host precomputes an int16 slot->pillar-row map M16 [16, S/16+8]
     (wrapped-by-16 layout); entry 0 = "no pillar" -> zero row.  (Building
     this on-device via indirect-DMA scatter was tried and reverted: HW
     SWDGE consumes only one offset per partition per instruction — see
     debug_scatter*.py — so a device-side map scatter costs 132 serial
     Pool instructions ~= 140us.)
  2. Load M16 into SBUF replicated across the 8 gpsimd core groups; groups
     4-7 get a one-window-shifted copy so one ap_gather call serves two
     windows (dual-window trick).
  3. DMA the padded feature table [PCAP, 64] f32 in, transpose on PE via
     identity matmuls into FT [128, PCAP] (channels on partitions,
     duplicated to both partition halves). Row 0 of the table is zeros ->
     FT column 0 is the zero column that unmapped slots gather.
  4. 24 gpsimd ap_gather calls: out_tile[c, j] = FT[c, M[slot]] builds the
     output in final channel-major layout, 2 windows (2*4464 slots) per call.
  5. Big contiguous DMA stores of [64, 4464] halves into out [64, S].
"""

import numpy as np

NX, NY = 432, 496
C = 64
S = NY * NX            # 214272
SW = S // 16           # 13392
MCOLS = SW + 8         # 13400 (8 dummy cols for padding-row writes)
PCAP = 16896           # feature table rows (128*132), row 0 = zero row
KCH = PCAP // 128      # 132
W = 4464               # slots per window (S = 48*W exactly)
NWIN = S // W          # 48
NPAIR = NWIN // 2      # 24
WCOL = W // 16         # 279
B = 8

_cache = {}


def _build_program():
    import concourse.bacc as bacc
    import concourse.tile as tile
    import concourse.mybir as mybir
    from concourse.masks import make_identity

    dt = mybir.dt
    nc = bacc.Bacc("TRN2", target_bir_lowering=False, debug=False, num_devices=B)

    feats = nc.dram_tensor("feats", [PCAP, C], dt.float32, kind="ExternalInput")
    m16 = nc.dram_tensor("m16", [16, MCOLS], dt.int16, kind="ExternalInput")
    out = nc.dram_tensor("out", [C, S], dt.float32, kind="ExternalOutput")

    with tile.TileContext(nc) as tc:
        with (
            tc.tile_pool(name="const", bufs=1) as cpool,
            tc.tile_pool(name="stage", bufs=2) as spool,
            tc.tile_pool(name="psum", bufs=4, space="PSUM") as ppool,
            tc.tile_pool(name="outp", bufs=2) as opool,
        ):
            # --- load map into SBUF: groups 0-3 plain, 4-7 shifted one window ---
            mt = cpool.tile([128, MCOLS], dt.int16)
            nc.vector.memset(mt[64:128, MCOLS - WCOL : MCOLS], 0)
            for a in range(8):
                if a < 4:
                    nc.sync.dma_start(
                        out=mt[16 * a : 16 * a + 16, 0:MCOLS], in_=m16.ap()
                    )
                else:
                    nc.sync.dma_start(
                        out=mt[16 * a : 16 * a + 16, 0 : MCOLS - WCOL],
                        in_=m16.ap()[:, WCOL:MCOLS],
                    )

            # --- feature table transpose: feats [PCAP, 64] -> FT [128, PCAP] ---
            ident = cpool.tile([128, 128], dt.float32)
            make_identity(nc, ident[:])
            ft = cpool.tile([128, PCAP], dt.float32)
            NG = 3
            GCH = KCH // NG  # 44 chunks of 128 rows per staging group
            feats_g = feats.ap().rearrange("(g n p) c -> g p n c", g=NG, p=128)
            for g in range(NG):
                st = spool.tile([128, GCH * C], dt.float32, tag="stage")
                nc.sync.dma_start(
                    out=st[:].rearrange("p (n c) -> p n c", c=C), in_=feats_g[g]
                )
                for q in range(GCH // 4):
                    pt = ppool.tile([64, 512], dt.float32, tag="pt")
                    for r in range(4):
                        n = q * 4 + r
                        nc.tensor.transpose(
                            out=pt[:, r * 128 : (r + 1) * 128],
                            in_=st[:, n * C : (n + 1) * C],
                            identity=ident[:],
                        )
                    col0 = (g * GCH + q * 4) * 128
                    nc.vector.tensor_copy(
                        out=ft[0:64, col0 : col0 + 512], in_=pt[:, :]
                    )
            nc.sync.dma_start(out=ft[64:128, :], in_=ft[0:64, :])

            # --- dual-window gathers + stores ---
            out_w = out.ap().rearrange("c (u w) -> u c w", w=W)
            for t in range(NPAIR):
                og = opool.tile([128, W], dt.float32, tag="og")
                nc.gpsimd.ap_gather(
                    out_ap=og[:, :],
                    in_ap=ft[:, :],
                    idxs_ap=mt[:, 2 * t * WCOL : (2 * t + 1) * WCOL],
                    channels=128,
                    num_elems=PCAP,
                    d=1,
                    num_idxs=W,
                )
                nc.sync.dma_start(out=out_w[2 * t], in_=og[0:64, :])
                nc.sync.dma_start(out=out_w[2 * t + 1], in_=og[64:128, :])

    nc.compile()
    return nc


def _get_program():
    if "nc" not in _cache:
        _cache["nc"] = _build_program()
    return _cache["nc"]


def _host_prep(pillar_features, coords_b, coords_z, coords_y, coords_x):
    """Shard pillars by batch; dedup last-wins; build per-core input maps."""
    feats = np.ascontiguousarray(np.asarray(pillar_features, dtype=np.float32))
    cb = np.asarray(coords_b).astype(np.int64)
    idx = (
        np.asarray(coords_z).astype(np.int64)
        + np.asarray(coords_y).astype(np.int64) * NX
        + np.asarray(coords_x).astype(np.int64)
    )

    in_maps = []
    for b in range(B):
        sel = np.nonzero(cb == b)[0]
        idx_b = idx[sel]
        # keep last occurrence per flat index (reference scatter semantics)
        rev = idx_b[::-1]
        uniq, pos_rev = np.unique(rev, return_index=True)
        keep = sel[len(idx_b) - 1 - pos_rev]  # global pillar ids, sorted by idx
        n = len(uniq)
        assert n + 1 <= PCAP, f"batch {b}: {n} unique pillars exceeds capacity"

        fp = np.zeros((PCAP, C), dtype=np.float32)
        fp[1 : n + 1] = feats[keep]

        m = np.zeros(16 * MCOLS, dtype=np.int16)
        m[(uniq % 16) * MCOLS + uniq // 16] = np.arange(1, n + 1, dtype=np.int16)
        in_maps.append({"feats": fp, "m16": m.reshape(16, MCOLS)})
    return in_maps


def kernel(pillar_features, coords_b, coords_z, coords_y, coords_x, batch_size):
    from concourse.bass_utils import run_bass_kernel_spmd

    assert int(batch_size) == B
    nc = _get_program()
    in_maps = _host_prep(pillar_features, coords_b, coords_z, coords_y, coords_x)
    res = run_bass_kernel_spmd(nc, in_maps, core_ids=list(range(B)), trace=False)
    out = np.empty((B, C, NY, NX), dtype=np.float32)
    for b in range(B):
        out[b] = res.results[b]["out"].reshape(C, NY, NX)
    return out


# revision 10
# speedup vs baseline: 7.4309x; 7.4309x over previous
"""PointPillarScatter on 8 Trainium2 NeuronCores.

out[b, c, y*NX+x] = pillar_features[p, c] for each pillar p with coords
(b, 0, y, x); duplicate (b,y,x) resolved last-pillar-wins; rest zeros.

Sharding: data-parallel over batch - core b handles batch element b.

Per-core device algorithm (selection-matmul formulation of the scatter):
  The BEV plane [C=64, S=214272] is split into 216 windows of W=992 slots.
  Host prep buckets each batch's deduped pillars by window into a packed
  feature table [216*128, 64] (window w's pillars at rows 128w..128w+n_w,
  zero-padded to 128; occupancy max ~101 < 128), plus an offset table
  idxloc[128, 216] giving each row's slot offset inside its window (-1 for
  padding rows).

  Per window on device:
    Sel[k, s]   = (idxloc[k, w] == s)          DVE is_equal vs an iota row
    psum[c, s]  = feats_block[k, c]^T @ Sel    PE matmul (fp32, exact: the
                                               one-hot column has a single
                                               nonzero so out = 1.0*value)
    stage       = psum                         ACT copy (PSUM -> SBUF)
    out[:, w*992:(w+1)*992] = stage            HWDGE DMA store

  This was chosen over a GPSIMD ap_gather formulation (measured ~29 ns per
  gathered slot-column -> ~3 ms/core) and over indirect-DMA scatter into the
  output layout (per-element strided writes). PE at fp32 does ~1.9 us per
  992-slot window -> ~400 us/core, with DVE/ACT/DMA all under that.
"""

import numpy as np

NX, NY = 432, 496
C = 64
S = NY * NX            # 214272
W = 992                # slots per window; S = 216*W exactly
NWIN = S // W          # 216
KP = 128               # padded pillar rows per window
TROWS = NWIN * KP      # 27648 feature-table rows
B = 8

_cache = {}


def _build_program():
    import concourse.bacc as bacc
    import concourse.tile as tile
    import concourse.mybir as mybir

    dt = mybir.dt
    nc = bacc.Bacc("TRN2", target_bir_lowering=False, debug=False, num_devices=B)

    feats = nc.dram_tensor("feats", [TROWS, C], dt.float32, kind="ExternalInput")
    idxloc = nc.dram_tensor("idxloc", [KP, NWIN], dt.float32, kind="ExternalInput")
    iota = nc.dram_tensor("iota", [KP, W], dt.float32, kind="ExternalInput")
    out = nc.dram_tensor("out", [C, S], dt.float32, kind="ExternalOutput")

    LB = 8  # windows of lhsT per staging load

    with tile.TileContext(nc) as tc:
        with (
            tc.tile_pool(name="const", bufs=1) as cpool,
            tc.tile_pool(name="lhs", bufs=3) as lpool,
            tc.tile_pool(name="sel", bufs=3) as selpool,
            tc.tile_pool(name="psum", bufs=4, space="PSUM") as ppool,
            tc.tile_pool(name="stg", bufs=3) as spool,
        ):
            iot = cpool.tile([KP, W], dt.float32)
            nc.sync.dma_start(out=iot[:], in_=iota.ap())
            idxt = cpool.tile([KP, NWIN], dt.float32)
            nc.sync.dma_start(out=idxt[:], in_=idxloc.ap())

            feats_b = feats.ap().rearrange("(g n p) c -> g p n c", g=NWIN // LB, p=KP)

            HW = W // 2  # 496 columns per matmul: one PSUM bank
            lt = None
            stg = None
            for w in range(NWIN):
                if w % LB == 0:
                    lt = lpool.tile([KP, LB * C], dt.float32, tag="lt")
                    nc.sync.dma_start(
                        out=lt[:].rearrange("p (n c) -> p n c", c=C),
                        in_=feats_b[w // LB],
                    )
                n = w % LB
                sel = selpool.tile([KP, W], dt.float32, tag="sel")
                nc.vector.tensor_tensor(
                    out=sel[:],
                    in0=iot[:],
                    in1=idxt[:, w : w + 1].to_broadcast([KP, W]),
                    op=mybir.AluOpType.is_equal,
                )
                if w % 2 == 0:
                    stg = spool.tile([C, 2 * W], dt.float32, tag="stg")
                for h in range(2):
                    pt = ppool.tile([C, HW], dt.float32, tag="pt")
                    nc.tensor.matmul(
                        out=pt[:],
                        lhsT=lt[:, n * C : (n + 1) * C],
                        rhs=sel[:, h * HW : (h + 1) * HW],
                        start=True,
                        stop=True,
                    )
                    nc.scalar.copy(
                        out=stg[:, (w % 2) * W + h * HW : (w % 2) * W + (h + 1) * HW],
                        in_=pt[:],
                    )
                if w % 2 == 1:
                    nc.sync.dma_start(
                        out=out.ap()[:, (w - 1) * W : (w + 1) * W], in_=stg[:]
                    )

    nc.compile()
    return nc


def _get_program():
    if "nc" not in _cache:
        _cache["nc"] = _build_program()
    return _cache["nc"]


def _host_prep(pillar_features, coords_b, coords_z, coords_y, coords_x):
    """Shard pillars by batch; dedup last-wins; pack per-window blocks."""
    feats = np.ascontiguousarray(np.asarray(pillar_features, dtype=np.float32))
    cb = np.asarray(coords_b).astype(np.int64)
    idx = (
        np.asarray(coords_z).astype(np.int64)
        + np.asarray(coords_y).astype(np.int64) * NX
        + np.asarray(coords_x).astype(np.int64)
    )

    iota = np.broadcast_to(np.arange(W, dtype=np.float32), (KP, W)).copy()

    in_maps = []
    for b in range(B):
        sel = np.nonzero(cb == b)[0]
        idx_b = idx[sel]
        # keep last occurrence per flat index (reference scatter semantics)
        rev = idx_b[::-1]
        uniq, pos_rev = np.unique(rev, return_index=True)
        keep = sel[len(idx_b) - 1 - pos_rev]  # pillar ids, sorted by idx

        wbin = uniq // W  # window of each pillar (sorted)
        woff = (uniq % W).astype(np.float32)
        counts = np.bincount(wbin, minlength=NWIN)
        assert counts.max() <= KP, f"batch {b}: window overflow {counts.max()}"
        starts = np.zeros(NWIN, np.int64)
        starts[1:] = np.cumsum(counts)[:-1]
        # packed row for pillar i (sorted order): 128*wbin + rank-in-window
        rank = np.arange(len(uniq)) - starts[wbin]
        rows = KP * wbin + rank

        fp = np.zeros((TROWS, C), dtype=np.float32)
        fp[rows] = feats[keep]
        il = np.full((NWIN, KP), -1.0, dtype=np.float32)
        il[wbin, rank] = woff
        in_maps.append({"feats": fp, "idxloc": il.T.copy(), "iota": iota})
    return in_maps


def kernel(pillar_features, coords_b, coords_z, coords_y, coords_x, batch_size):
    from concourse.bass_utils import run_bass_kernel_spmd

    assert int(batch_size) == B
    nc = _get_program()
    in_maps = _host_prep(pillar_features, coords_b, coords_z, coords_y, coords_x)
    res = run_bass_kernel_spmd(nc, in_maps, core_ids=list(range(B)), trace=False)
    out = np.empty((B, C, NY, NX), dtype=np.float32)
    for b in range(B):
        out[b] = res.results[b]["out"].reshape(C, NY, NX)
    return out


# revision 12
# speedup vs baseline: 9.7194x; 1.3080x over previous
"""PointPillarScatter on 8 Trainium2 NeuronCores.

out[b, c, y*NX+x] = pillar_features[p, c] for each pillar p with coords
(b, 0, y, x); duplicate (b,y,x) resolved last-pillar-wins; rest zeros.

Sharding: data-parallel over batch - core b handles batch element b.

Per-core device algorithm (selection-matmul formulation of the scatter):
  The BEV plane [C=64, S=214272] is split into 432 windows of W=496 slots
  (496 f32 = one PSUM bank).  Windows are processed in pairs (t, t+216)
  packed block-diagonally so one matmul emits both:

    lhsT[128, 128]: rows 0:64   = window t      pillars, feature cols 0:64
                    rows 64:128 = window t+216  pillars, feature cols 64:128
    Sel[k, s] = (idxloc[k] == s)   one-hot over the window's 496 slots
    psum[128, 496] = lhsT^T @ Sel  -> partitions 0:64 = window t [C, 496],
                                      partitions 64:128 = window t+216

  fp32 matmul with a one-hot rhs is exact (each output column has a single
  1.0 term).  ACT copies PSUM->SBUF; stores write [64, 1984] contiguous
  slabs per output half.  Host prep dedups (last-wins, matching the
  reference scatter), buckets pillars by window (max occupancy 58 < 64),
  and packs the block-diagonal table.

  Measured alternatives this replaces: GPSIMD ap_gather assembly (~29 ns
  per slot-column -> ~3 ms/core) and unpaired [64, 496] matmuls (~400 us,
  PE- and ACT-bound at M=64 = half the PE array).
"""

import numpy as np

NX, NY = 432, 496
C = 64
S = NY * NX            # 214272
W = 496                # slots per window = one PSUM bank of f32
NWIN = S // W          # 432
NPAIR = NWIN // 2      # 216 block-diagonal pairs: (t, t+216)
KH = 64                # max pillars per window (half of the 128 k rows)
TROWS = NPAIR * 128    # 27648 table rows
B = 8

_cache = {}


def _build_program():
    import concourse.bacc as bacc
    import concourse.tile as tile
    import concourse.mybir as mybir

    dt = mybir.dt
    nc = bacc.Bacc("TRN2", target_bir_lowering=False, debug=False, num_devices=B)

    feats = nc.dram_tensor("feats", [TROWS, 2 * C], dt.float32, kind="ExternalInput")
    idxloc = nc.dram_tensor("idxloc", [128, NPAIR], dt.float32, kind="ExternalInput")
    iota = nc.dram_tensor("iota", [128, W], dt.float32, kind="ExternalInput")
    out = nc.dram_tensor("out", [C, S], dt.float32, kind="ExternalOutput")

    LB = 8   # pairs of lhsT per staging load
    SG = 4   # pairs per store stage: [64, 4*496] slabs per half

    with tile.TileContext(nc) as tc:
        with (
            tc.tile_pool(name="const", bufs=1) as cpool,
            tc.tile_pool(name="lhs", bufs=3) as lpool,
            tc.tile_pool(name="sel", bufs=4) as selpool,
            tc.tile_pool(name="psum", bufs=4, space="PSUM") as ppool,
            tc.tile_pool(name="stg", bufs=3) as spool,
        ):
            iot = cpool.tile([128, W], dt.float32)
            nc.sync.dma_start(out=iot[:], in_=iota.ap())
            idxt = cpool.tile([128, NPAIR], dt.float32)
            nc.sync.dma_start(out=idxt[:], in_=idxloc.ap())

            feats_b = feats.ap().rearrange(
                "(g n p) c -> g p n c", g=NPAIR // LB, p=128
            )

            lt = None
            stg = None
            for t in range(NPAIR):
                if t % LB == 0:
                    lt = lpool.tile([128, LB * 2 * C], dt.float32, tag="lt")
                    nc.sync.dma_start(
                        out=lt[:].rearrange("p (n c) -> p n c", c=2 * C),
                        in_=feats_b[t // LB],
                    )
                n = t % LB
                sel = selpool.tile([128, W], dt.float32, tag="sel")
                nc.vector.tensor_tensor(
                    out=sel[:],
                    in0=iot[:],
                    in1=idxt[:, t : t + 1].to_broadcast([128, W]),
                    op=mybir.AluOpType.is_equal,
                )
                pt = ppool.tile([128, W], dt.float32, tag="pt")
                nc.tensor.matmul(
                    out=pt[:],
                    lhsT=lt[:, n * 2 * C : (n + 1) * 2 * C],
                    rhs=sel[:],
                    start=True,
                    stop=True,
                )
                u = t % SG
                if u == 0:
                    stg = spool.tile([128, SG * W], dt.float32, tag="stg")
                nc.scalar.copy(out=stg[:, u * W : (u + 1) * W], in_=pt[:])
                if u == SG - 1:
                    t0 = t - (SG - 1)
                    nc.sync.dma_start(
                        out=out.ap()[:, t0 * W : (t0 + SG) * W], in_=stg[0:64, :]
                    )
                    nc.sync.dma_start(
                        out=out.ap()[:, (NPAIR + t0) * W : (NPAIR + t0 + SG) * W],
                        in_=stg[64:128, :],
                    )

    nc.compile()
    return nc


def _get_program():
    if "nc" not in _cache:
        _cache["nc"] = _build_program()
    return _cache["nc"]


def _host_prep(pillar_features, coords_b, coords_z, coords_y, coords_x):
    """Shard pillars by batch; dedup last-wins; pack block-diagonal pairs."""
    feats = np.ascontiguousarray(np.asarray(pillar_features, dtype=np.float32))
    cb = np.asarray(coords_b).astype(np.int64)
    idx = (
        np.asarray(coords_z).astype(np.int64)
        + np.asarray(coords_y).astype(np.int64) * NX
        + np.asarray(coords_x).astype(np.int64)
    )

    iota = np.broadcast_to(np.arange(W, dtype=np.float32), (128, W)).copy()

    in_maps = []
    for b in range(B):
        sel = np.nonzero(cb == b)[0]
        idx_b = idx[sel]
        # keep last occurrence per flat index (reference scatter semantics)
        rev = idx_b[::-1]
        uniq, pos_rev = np.unique(rev, return_index=True)
        keep = sel[len(idx_b) - 1 - pos_rev]  # pillar ids, sorted by idx

        wbin = uniq // W                       # window id, sorted
        woff = (uniq % W).astype(np.float32)
        counts = np.bincount(wbin, minlength=NWIN)
        assert counts.max() <= KH, f"batch {b}: window overflow {counts.max()}"
        starts = np.zeros(NWIN, np.int64)
        starts[1:] = np.cumsum(counts)[:-1]
        rank = np.arange(len(uniq)) - starts[wbin]
        # window w pairs into block t = w % NPAIR, half h = w // NPAIR
        blk = wbin % NPAIR
        half = wbin // NPAIR
        rows = 128 * blk + KH * half + rank    # table row of each pillar
        fp = np.zeros((TROWS, 2 * C), dtype=np.float32)
        fp[rows[:, None], (half * C)[:, None] + np.arange(C)[None, :]] = feats[keep]
        il = np.full((NPAIR, 128), -1.0, dtype=np.float32)
        il[blk, KH * half + rank] = woff
        in_maps.append({"feats": fp, "idxloc": il.T.copy(), "iota": iota})
    return in_maps


def kernel(pillar_features, coords_b, coords_z, coords_y, coords_x, batch_size):
    from concourse.bass_utils import run_bass_kernel_spmd

    assert int(batch_size) == B
    nc = _get_program()
    in_maps = _host_prep(pillar_features, coords_b, coords_z, coords_y, coords_x)
    res = run_bass_kernel_spmd(nc, in_maps, core_ids=list(range(B)), trace=False)
    out = np.empty((B, C, NY, NX), dtype=np.float32)
    for b in range(B):
        out[b] = res.results[b]["out"].reshape(C, NY, NX)
    return out


# revision 13
# speedup vs baseline: 9.7645x; 1.0046x over previous
"""PointPillarScatter on 8 Trainium2 NeuronCores.

out[b, c, y*NX+x] = pillar_features[p, c] for each pillar p with coords
(b, 0, y, x); duplicate (b,y,x) resolved last-pillar-wins; rest zeros.

Sharding: data-parallel over batch - core b handles batch element b.

Per-core device algorithm (selection-matmul formulation of the scatter):
  The BEV plane [C=64, S=214272] is split into 432 windows of W=496 slots
  (496 f32 = one PSUM bank).  Windows are processed in pairs (t, t+216)
  packed block-diagonally so one matmul emits both:

    lhsT[128, 128]: rows 0:64   = window t      pillars, feature cols 0:64
                    rows 64:128 = window t+216  pillars, feature cols 64:128
    Sel[k, s] = (idxloc[k] == s)   one-hot over the window's 496 slots
    psum[128, 496] = lhsT^T @ Sel  -> partitions 0:64 = window t [C, 496],
                                      partitions 64:128 = window t+216

  fp32 matmul with a one-hot rhs is exact (each output column has a single
  1.0 term).  ACT copies PSUM->SBUF; stores write [64, 1984] contiguous
  slabs per output half.  Host prep dedups (last-wins, matching the
  reference scatter), buckets pillars by window (max occupancy 58 < 64),
  and packs the block-diagonal table.

  Measured alternatives this replaces: GPSIMD ap_gather assembly (~29 ns
  per slot-column -> ~3 ms/core) and unpaired [64, 496] matmuls (~400 us,
  PE- and ACT-bound at M=64 = half the PE array).
"""

import numpy as np

NX, NY = 432, 496
C = 64
S = NY * NX            # 214272
W = 496                # slots per window = one PSUM bank of f32
NWIN = S // W          # 432
NPAIR = NWIN // 2      # 216 block-diagonal pairs: (t, t+216)
KH = 64                # max pillars per window (half of the 128 k rows)
TROWS = NPAIR * 128    # 27648 table rows
B = 8

_cache = {}


def _build_program():
    import concourse.bacc as bacc
    import concourse.tile as tile
    import concourse.mybir as mybir

    dt = mybir.dt
    nc = bacc.Bacc("TRN2", target_bir_lowering=False, debug=False, num_devices=B)

    feats = nc.dram_tensor("feats", [TROWS, 2 * C], dt.float32, kind="ExternalInput")
    idxloc = nc.dram_tensor("idxloc", [128, NPAIR], dt.float32, kind="ExternalInput")
    iota = nc.dram_tensor("iota", [128, W], dt.float32, kind="ExternalInput")
    out = nc.dram_tensor("out", [C, S], dt.float32, kind="ExternalOutput")

    LB = 8   # pairs of lhsT per staging load
    SG = 4   # pairs per store stage: [64, 4*496] slabs per half

    with tile.TileContext(nc) as tc:
        with (
            tc.tile_pool(name="const", bufs=1) as cpool,
            tc.tile_pool(name="lhs", bufs=3) as lpool,
            tc.tile_pool(name="sel", bufs=6) as selpool,
            tc.tile_pool(name="psum", bufs=8, space="PSUM") as ppool,
            tc.tile_pool(name="stg", bufs=3) as spool,
        ):
            iot = cpool.tile([128, W], dt.float32)
            nc.sync.dma_start(out=iot[:], in_=iota.ap())
            idxt = cpool.tile([128, NPAIR], dt.float32)
            nc.sync.dma_start(out=idxt[:], in_=idxloc.ap())

            feats_b = feats.ap().rearrange(
                "(g n p) c -> g p n c", g=NPAIR // LB, p=128
            )

            lt = None
            stg = None
            for t in range(NPAIR):
                if t % LB == 0:
                    lt = lpool.tile([128, LB * 2 * C], dt.float32, tag="lt")
                    nc.sync.dma_start(
                        out=lt[:].rearrange("p (n c) -> p n c", c=2 * C),
                        in_=feats_b[t // LB],
                    )
                n = t % LB
                sel = selpool.tile([128, W], dt.float32, tag="sel")
                nc.vector.tensor_tensor(
                    out=sel[:],
                    in0=iot[:],
                    in1=idxt[:, t : t + 1].to_broadcast([128, W]),
                    op=mybir.AluOpType.is_equal,
                )
                pt = ppool.tile([128, W], dt.float32, tag="pt")
                nc.tensor.matmul(
                    out=pt[:],
                    lhsT=lt[:, n * 2 * C : (n + 1) * 2 * C],
                    rhs=sel[:],
                    start=True,
                    stop=True,
                )
                u = t % SG
                if u == 0:
                    stg = spool.tile([128, SG * W], dt.float32, tag="stg")
                nc.scalar.copy(out=stg[:, u * W : (u + 1) * W], in_=pt[:])
                if u == SG - 1:
                    t0 = t - (SG - 1)
                    nc.sync.dma_start(
                        out=out.ap()[:, t0 * W : (t0 + SG) * W], in_=stg[0:64, :]
                    )
                    nc.sync.dma_start(
                        out=out.ap()[:, (NPAIR + t0) * W : (NPAIR + t0 + SG) * W],
                        in_=stg[64:128, :],
                    )

    nc.compile()
    return nc


def _get_program():
    if "nc" not in _cache:
        _cache["nc"] = _build_program()
    return _cache["nc"]


def _host_prep(pillar_features, coords_b, coords_z, coords_y, coords_x):
    """Shard pillars by batch; dedup last-wins; pack block-diagonal pairs."""
    feats = np.ascontiguousarray(np.asarray(pillar_features, dtype=np.float32))
    cb = np.asarray(coords_b).astype(np.int64)
    idx = (
        np.asarray(coords_z).astype(np.int64)
        + np.asarray(coords_y).astype(np.int64) * NX
        + np.asarray(coords_x).astype(np.int64)
    )

    iota = np.broadcast_to(np.arange(W, dtype=np.float32), (128, W)).copy()

    in_maps = []
    for b in range(B):
        sel = np.nonzero(cb == b)[0]
        idx_b = idx[sel]
        # keep last occurrence per flat index (reference scatter semantics)
        rev = idx_b[::-1]
        uniq, pos_rev = np.unique(rev, return_index=True)
        keep = sel[len(idx_b) - 1 - pos_rev]  # pillar ids, sorted by idx

        wbin = uniq // W                       # window id, sorted
        woff = (uniq % W).astype(np.float32)
        counts = np.bincount(wbin, minlength=NWIN)
        assert counts.max() <= KH, f"batch {b}: window overflow {counts.max()}"
        starts = np.zeros(NWIN, np.int64)
        starts[1:] = np.cumsum(counts)[:-1]
        rank = np.arange(len(uniq)) - starts[wbin]
        # window w pairs into block t = w % NPAIR, half h = w // NPAIR
        blk = wbin % NPAIR
        half = wbin // NPAIR
        rows = 128 * blk + KH * half + rank    # table row of each pillar
        fp = np.zeros((TROWS, 2 * C), dtype=np.float32)
        fp[rows[:, None], (half * C)[:, None] + np.arange(C)[None, :]] = feats[keep]
        il = np.full((NPAIR, 128), -1.0, dtype=np.float32)
        il[blk, KH * half + rank] = woff
        in_maps.append({"feats": fp, "idxloc": il.T.copy(), "iota": iota})
    return in_maps


def kernel(pillar_features, coords_b, coords_z, coords_y, coords_x, batch_size):
    from concourse.bass_utils import run_bass_kernel_spmd

    assert int(batch_size) == B
    nc = _get_program()
    in_maps = _host_prep(pillar_features, coords_b, coords_z, coords_y, coords_x)
    res = run_bass_kernel_spmd(nc, in_maps, core_ids=list(range(B)), trace=False)
    out = np.empty((B, C, NY, NX), dtype=np.float32)
    for b in range(B):
        out[b] = res.results[b]["out"].reshape(C, NY, NX)
    return out
